# revision 4
# baseline (speedup 1.0000x reference)
"""Causal segment-masked depthwise conv (K=5) + pointwise conv, 8-core SPMD.

Strategy:
  Host: pack each batch row's segments into a global stream with 4 zeros
  before each segment (plain causal conv on the stream == per-segment
  left-zero-padded conv), split the stream evenly across 8 cores with a
  4-element halo, and pre-transpose each core's slab to [C, stream].
  Device: depthwise conv as diag-stationary matmuls (shifts along the free
  dim are AP offsets) accumulated in PSUM, ACT copy + b_dw bias, pointwise
  matmul (dwT chunks stationary, w_pw^T moving -> natural [l, d] output),
  DVE adds b_pw, store. All matmuls in float32r (full-rate fp32).
"""

import sys

sys.path.insert(0, "/opt/trn_rl_repo")

import numpy as np

B, L, C, K, S = 8, 4096, 512, 5, 8
NCORES = 8
CCH = C // 128          # 4 channel chunks
XT_W = 4360             # per-core stream buffer: 4 halo + 4348 capacity + pad
OUT_ROWS = 4352         # conv outputs for stream cols [4, 4356)
NBLK = 9                # 8 blocks of 512 + 1 of 256
BLKS = [512] * 8 + [256]

_cached = {}


def _build_nc():
    import concourse.mybir as mybir
    from concourse import bacc
    from concourse.tile import TileContext
    from concourse.masks import make_identity

    f32 = mybir.dt.float32
    f32r = mybir.dt.float32r

    nc = bacc.Bacc()
    xt_d = nc.declare_dram_parameter("xt", [C, XT_W], f32, isOutput=False)
    wdiag_d = nc.declare_dram_parameter("wdiag", [128, CCH * K], f32, isOutput=False)
    wpwt_d = nc.declare_dram_parameter("wpwt", [128, CCH, C], f32, isOutput=False)
    bdw_d = nc.declare_dram_parameter("bdw", [128, CCH], f32, isOutput=False)
    bpwr_d = nc.declare_dram_parameter("bpwr", [128, C], f32, isOutput=False)
    out_d = nc.declare_dram_parameter("out", [OUT_ROWS, C], f32, isOutput=True)

    with TileContext(nc) as tc:
        with (
            tc.tile_pool(name="consts", bufs=1) as cpool,
            tc.tile_pool(name="xt", bufs=2) as xt_pool,
            tc.tile_pool(name="dwt", bufs=CCH * NBLK) as dwt_pool,
            tc.tile_pool(name="outsb", bufs=4) as out_pool,
            tc.tile_pool(name="dwps", bufs=2, space="PSUM") as dw_psum,
            tc.tile_pool(name="outps", bufs=2, space="PSUM") as out_psum,
        ):
            # constants
            wdiag_src = cpool.tile([128, CCH * K], f32)
            nc.sync.dma_start(out=wdiag_src[:], in_=wdiag_d[:])
            wpwt = cpool.tile([128, CCH, C], f32r)
            nc.gpsimd.dma_start(out=wpwt[:], in_=wpwt_d[:])
            bdw = cpool.tile([128, CCH], f32)
            nc.sync.dma_start(out=bdw[:], in_=bdw_d[:])
            bpwr = cpool.tile([128, C], f32)
            nc.sync.dma_start(out=bpwr[:], in_=bpwr_d[:])

            ident = cpool.tile([128, 128], f32)
            make_identity(nc, ident[:])
            diag = cpool.tile([128, CCH * K * 128], f32r)
            for u in range(CCH * K):
                nc.vector.tensor_scalar_mul(
                    diag[:, u * 128 : (u + 1) * 128],
                    ident[:],
                    wdiag_src[:, u : u + 1],
                )

            dwt = [[None] * NBLK for _ in range(CCH)]

            def conv_chunk(j):
                xtj = xt_pool.tile([128, XT_W], f32r)
                nc.gpsimd.dma_start(out=xtj[:], in_=xt_d[j * 128 : (j + 1) * 128, :])
                for kb in range(NBLK):
                    blk = BLKS[kb]
                    base = 4 + 512 * kb
                    ps = dw_psum.tile([128, blk], f32, tag="dwps")
                    for k in range(K):
                        # dwT[c, i] = sum_k w_dw[c, k] * xt[c, i - (4 - k)]
                        off = base - (4 - k)
                        nc.tensor.matmul(
                            ps[:],
                            lhsT=diag[:, (j * K + k) * 128 : (j * K + k + 1) * 128],
                            rhs=xtj[:, off : off + blk],
                            start=(k == 0),
                            stop=(k == K - 1),
                        )
                    dt_ = dwt_pool.tile([128, blk], f32r, tag="dwt")
                    # PSUM -> SBUF with per-partition b_dw bias (ACT engine)
                    nc.scalar.add(dt_[:], ps[:], bdw[:, j : j + 1])
                    dwt[j][kb] = dt_

            def pointwise(kb):
                blk = BLKS[kb]
                for t in range(blk // 128):
                    po = out_psum.tile([128, C], f32, tag="outps")
                    for j in range(CCH):
                        nc.tensor.matmul(
                            po[:],
                            lhsT=dwt[j][kb][:, t * 128 : (t + 1) * 128],
                            rhs=wpwt[:, j, :],
                            start=(j == 0),
                            stop=(j == CCH - 1),
                        )
                    ob = out_pool.tile([128, C], f32, tag="outsb")
                    nc.vector.tensor_add(ob[:], po[:], bpwr[:])
                    r0 = 512 * kb + t * 128
                    nc.sync.dma_start(out=out_d[r0 : r0 + 128, :], in_=ob[:])

            for j in range(CCH - 1):
                conv_chunk(j)
            conv_chunk(CCH - 1)
            for kb in range(NBLK):
                pointwise(kb)

    nc.finalize()
    return nc


def _get_nc():
    if "nc" not in _cached:
        _cached["nc"] = _build_nc()
    return _cached["nc"]


def _analyze(segment_boundaries):
    starts = segment_boundaries[..., 0].astype(np.int64)  # [B,S]
    ends = segment_boundaries[..., 1].astype(np.int64)
    pos = np.arange(L)
    in_seg = (pos[None, None, :] >= starts[..., None]) & (
        pos[None, None, :] < ends[..., None]
    )  # [B,S,L]
    covered = in_seg.any(axis=1)
    seg_id = np.where(covered, in_seg.argmax(axis=1), -1)  # [B,L]
    return covered, seg_id


def kernel(x, segment_boundaries, w_dw, b_dw, w_pw, b_pw):
    from concourse.bass_utils import run_bass_kernel_spmd

    x = np.asarray(x, dtype=np.float32)
    sb = np.asarray(segment_boundaries)
    w_dw = np.asarray(w_dw, dtype=np.float32)
    b_dw = np.asarray(b_dw, dtype=np.float32)
    w_pw = np.asarray(w_pw, dtype=np.float32)
    b_pw = np.asarray(b_pw, dtype=np.float32)

    covered, seg_id = _analyze(sb)

    # ---- run decomposition + stream build ----
    pieces = []          # [len, C] chunks
    src_b_parts = []
    src_l_parts = []
    run_start_of = np.full((B, L), -1, np.int64)  # run start index per covered pos
    for b in range(B):
        sid = seg_id[b]
        change = np.nonzero(np.diff(sid) != 0)[0] + 1
        bounds = np.concatenate([[0], change, [L]])
        for s, e in zip(bounds[:-1], bounds[1:]):
            if sid[s] < 0:
                continue
            run_start_of[b, s:e] = s
            pieces.append(np.zeros((4, C), np.float32))
            src_b_parts.append(np.full(4, -1, np.int64))
            src_l_parts.append(np.full(4, -1, np.int64))
            pieces.append(x[b, s:e])
            src_b_parts.append(np.full(e - s, b, np.int64))
            src_l_parts.append(np.arange(s, e, dtype=np.int64))
    if pieces:
        stream = np.concatenate(pieces, axis=0)
        src_b = np.concatenate(src_b_parts)
        src_l = np.concatenate(src_l_parts)
    else:
        stream = np.zeros((0, C), np.float32)
        src_b = np.zeros(0, np.int64)
        src_l = np.zeros(0, np.int64)
    T = stream.shape[0]
    Q = -(-T // NCORES) if T else 1
    assert Q + 4 <= XT_W - 8, f"stream quota {Q} too large"

    # ---- per-core inputs ----
    wdiag = np.ascontiguousarray(
        w_dw.reshape(CCH, 128, K).transpose(1, 0, 2).reshape(128, CCH * K)
    )
    wpwt = np.ascontiguousarray(
        w_pw.T.reshape(CCH, 128, C).transpose(1, 0, 2)
    )  # [128, j, d] = w_pw[d, j*128+p]
    bdwr = np.ascontiguousarray(b_dw.reshape(CCH, 128).T)  # [128, CCH]
    bpwr = np.ascontiguousarray(np.broadcast_to(b_pw[None, :], (128, C)))

    in_maps = []
    spans = []
    for i in range(NCORES):
        lo, hi = i * Q, min((i + 1) * Q, T)
        lo = min(lo, T)
        spans.append((lo, hi))
        buf = np.zeros((XT_W, C), np.float32)
        if hi > lo:
            hlo = max(0, lo - 4)
            buf[4 - (lo - hlo) : 4 + (hi - lo)] = stream[hlo:hi]
        in_maps.append(
            {
                "xt": np.ascontiguousarray(buf.T),
                "wdiag": wdiag,
                "wpwt": wpwt,
                "bdw": bdwr,
                "bpwr": bpwr,
            }
        )

    nc = _get_nc()
    res = run_bass_kernel_spmd(nc, in_maps, list(range(NCORES)))

    # ---- gather ----
    so_out = np.zeros((T, C), np.float32)
    for i, (lo, hi) in enumerate(spans):
        if hi > lo:
            so_out[lo:hi] = res.results[i]["out"][: hi - lo]
    out = np.zeros((B, L, C), np.float32)
    mask = src_l >= 0
    out[src_b[mask], src_l[mask]] = so_out[mask]

    # ---- general-case sparse correction (pairwise mask vs run mask) ----
    # reference: m_ref_d[l] = covered[l] & l>=d & seg_id[l-d]==seg_id[l]
    # device computed run mask: m_run_d[l] = covered[l] & (l - run_start >= d)
    need = []
    for d in range(1, K):
        m_ref = np.zeros((B, L), bool)
        m_ref[:, d:] = covered[:, d:] & (seg_id[:, d:] == seg_id[:, :-d])
        m_run = covered & (np.arange(L)[None, :] - run_start_of >= d)
        diff = m_ref.astype(np.int8) - m_run.astype(np.int8)
        if np.any(diff):
            bs, ls = np.nonzero(diff)
            need.append((d, bs, ls, diff[bs, ls].astype(np.float32)))
    if need:
        for d, bs, ls, sgn in need:
            xv = x[bs, ls - d, :]  # ls >= d guaranteed where masks differ
            delta_dw = xv * w_dw[None, :, K - 1 - d] * sgn[:, None]
            out[bs, ls, :] += delta_dw @ w_pw.T

    return out


# revision 9
# speedup vs baseline: 1.2686x; 1.2686x over previous
"""Causal segment-masked depthwise conv (K=5) + pointwise conv, 8-core SPMD.

Strategy:
  Host: pack each batch row's segments into a global stream with 4 zeros
  before each segment (plain causal conv on the stream == per-segment
  left-zero-padded conv), split the stream evenly across 8 cores with a
  4-element halo, and pre-transpose each core's slab to [C, stream].
  Device (per 512-col block): depthwise conv for channel chunks 0-1 as
  diag-stationary fp32r matmuls in PSUM (+ ACT bias copy), chunks 2-3 on
  DVE via fused scalar_tensor_tensor; pointwise matmul with w_pw^T chunks
  stationary and dwT moving -> transposed [d, l] PSUM output, ACT adds
  b_pw as per-partition bias, store. Host transposes back during gather.
"""

import os
import sys

sys.path.insert(0, "/opt/trn_rl_repo")

import numpy as np

_STORE_ENG = os.environ.get("KSTORE", "scalar")   # scalar | sync
_LOAD_MODE = os.environ.get("KLOAD", "split")     # split | sync
_PE_CH_ENV = int(os.environ.get("KPECH", "2"))    # conv chunks on PE

B, L, C, K, S = 8, 4096, 512, 5, 8
NCORES = 8
CCH = C // 128          # 4 channel chunks
XT_W = 4360             # per-core stream buffer: 4 halo + 4348 capacity + pad
OUT_ROWS = 4352         # conv outputs for stream cols [4, 4356)
NBLK = 9                # 8 blocks of 512 + 1 of 256
BLKS = [512] * 8 + [256]
PE_CH = _PE_CH_ENV      # channel chunks 0..PE_CH-1 on PE, rest on DVE

_cached = {}


def _build_nc():
    import concourse.mybir as mybir
    from concourse import bacc
    from concourse.tile import TileContext
    from concourse.masks import make_identity

    f32 = mybir.dt.float32
    f32r = mybir.dt.float32r
    Alu = mybir.AluOpType

    nc = bacc.Bacc()
    xt_d = nc.declare_dram_parameter("xt", [C, XT_W], f32, isOutput=False)
    wdiag_d = nc.declare_dram_parameter("wdiag", [128, CCH * K], f32, isOutput=False)
    wpwt_d = nc.declare_dram_parameter(
        "wpwt", [128, CCH, CCH, 128], f32, isOutput=False
    )
    bdw_d = nc.declare_dram_parameter("bdw", [128, CCH], f32, isOutput=False)
    bpw_d = nc.declare_dram_parameter("bpw", [128, CCH], f32, isOutput=False)
    out_d = nc.declare_dram_parameter("out", [C, OUT_ROWS], f32, isOutput=True)

    with TileContext(nc) as tc:
        with (
            tc.tile_pool(name="consts", bufs=1) as cpool,
            tc.tile_pool(name="xt", bufs=3) as xt_pool,
            tc.tile_pool(name="dwt", bufs=3) as dwt_pool,
            tc.tile_pool(name="outsb", bufs=6) as out_pool,
            tc.tile_pool(name="dwps", bufs=3, space="PSUM") as dw_psum,
            tc.tile_pool(name="outps", bufs=3, space="PSUM") as out_psum,
        ):
            # constants (small, sync ring, first)
            wdiag_src = cpool.tile([128, CCH * K], f32)
            nc.sync.dma_start(out=wdiag_src[:], in_=wdiag_d[:])
            bdw = cpool.tile([128, CCH], f32)
            nc.sync.dma_start(out=bdw[:], in_=bdw_d[:])
            bpw = cpool.tile([128, CCH], f32)
            nc.sync.dma_start(out=bpw[:], in_=bpw_d[:])
            wpwt = cpool.tile([128, CCH, CCH, 128], f32r)
            nc.gpsimd.dma_start(out=wpwt[:], in_=wpwt_d[:])

            ident = cpool.tile([128, 128], f32)
            make_identity(nc, ident[:])
            diag = cpool.tile([128, PE_CH * K * 128], f32r)
            for u in range(PE_CH * K):
                nc.vector.tensor_scalar_mul(
                    diag[:, u * 128 : (u + 1) * 128],
                    ident[:],
                    wdiag_src[:, u : u + 1],
                )

            for lb in range(NBLK):
                blk = BLKS[lb]
                w = blk + 8  # 4 halo + 4 pad for alignment
                xts = []
                for j in range(CCH):
                    xtj = xt_pool.tile(
                        [128, w], f32r if j < PE_CH else f32, tag=f"xt{j}", name=f"xt{j}_{lb}"
                    )
                    src = xt_d[j * 128 : (j + 1) * 128, 512 * lb : 512 * lb + blk + 4]
                    if j < PE_CH and _LOAD_MODE == "split":
                        nc.gpsimd.dma_start(out=xtj[:, 0 : blk + 4], in_=src)
                    elif j < PE_CH:
                        nc.gpsimd.dma_start(out=xtj[:, 0 : blk + 4], in_=src)
                    else:
                        nc.sync.dma_start(out=xtj[:, 0 : blk + 4], in_=src)
                    xts.append(xtj)

                dwt = []
                # conv: PE chunks
                for j in range(PE_CH):
                    ps = dw_psum.tile([128, blk], f32, tag="dwps", name=f"ps{j}_{lb}")
                    for k in range(K):
                        # dwT[c, t] = sum_k w_dw[c, k] * xt[c, t + k]  (tile-local)
                        nc.tensor.matmul(
                            ps[:],
                            lhsT=diag[:, (j * K + k) * 128 : (j * K + k + 1) * 128],
                            rhs=xts[j][:, k : k + blk],
                            start=(k == 0),
                            stop=(k == K - 1),
                        )
                    dt_ = dwt_pool.tile([128, blk], f32r, tag=f"dwt{j}", name=f"dwt{j}_{lb}")
                    nc.scalar.add(dt_[:], ps[:], bdw[:, j : j + 1])
                    dwt.append(dt_)
                # conv: DVE chunks, fused multiply-accumulate
                for j in range(PE_CH, CCH):
                    dt_ = dwt_pool.tile([128, blk], f32r, tag=f"dwt{j}", name=f"dwt{j}_{lb}")
                    nc.vector.tensor_scalar(
                        dt_[:],
                        xts[j][:, 0:blk],
                        wdiag_src[:, j * K : j * K + 1],
                        bdw[:, j : j + 1],
                        op0=Alu.mult,
                        op1=Alu.add,
                    )
                    for k in range(1, K):
                        nc.vector.scalar_tensor_tensor(
                            dt_[:],
                            xts[j][:, k : k + blk],
                            wdiag_src[:, j * K + k : j * K + k + 1],
                            dt_[:],
                            op0=Alu.mult,
                            op1=Alu.add,
                        )
                    dwt.append(dt_)

                # pointwise: outT[d, l] += w_pwT[c, d].T @ dwT[c, l]
                for dch in range(CCH):
                    po = out_psum.tile([128, blk], f32, tag="outps", name=f"po{dch}_{lb}")
                    for j in range(CCH):
                        nc.tensor.matmul(
                            po[:],
                            lhsT=wpwt[:, j, dch, :],
                            rhs=dwt[j][:, 0:blk],
                            start=(j == 0),
                            stop=(j == CCH - 1),
                        )
                    ob = out_pool.tile([128, blk], f32, tag="outsb", name=f"ob{dch}_{lb}")
                    nc.scalar.add(ob[:], po[:], bpw[:, dch : dch + 1])
                    st_eng = nc.scalar if _STORE_ENG == "scalar" else nc.sync
                    st_eng.dma_start(
                        out=out_d[dch * 128 : (dch + 1) * 128, 512 * lb : 512 * lb + blk],
                        in_=ob[:],
                    )

    nc.finalize()
    return nc


def _get_nc():
    if "nc" not in _cached:
        _cached["nc"] = _build_nc()
    return _cached["nc"]


def _analyze(segment_boundaries):
    starts = segment_boundaries[..., 0].astype(np.int64)  # [B,S]
    ends = segment_boundaries[..., 1].astype(np.int64)
    pos = np.arange(L)
    in_seg = (pos[None, None, :] >= starts[..., None]) & (
        pos[None, None, :] < ends[..., None]
    )  # [B,S,L]
    covered = in_seg.any(axis=1)
    seg_id = np.where(covered, in_seg.argmax(axis=1), -1)  # [B,L]
    return covered, seg_id


def kernel(x, segment_boundaries, w_dw, b_dw, w_pw, b_pw):
    from concourse.bass_utils import run_bass_kernel_spmd

    x = np.asarray(x, dtype=np.float32)
    sb = np.asarray(segment_boundaries)
    w_dw = np.asarray(w_dw, dtype=np.float32)
    b_dw = np.asarray(b_dw, dtype=np.float32)
    w_pw = np.asarray(w_pw, dtype=np.float32)
    b_pw = np.asarray(b_pw, dtype=np.float32)

    covered, seg_id = _analyze(sb)

    # ---- run decomposition + stream build ----
    pieces = []          # [len, C] chunks
    src_b_parts = []
    src_l_parts = []
    run_start_of = np.full((B, L), -1, np.int64)  # run start index per covered pos
    for b in range(B):
        sid = seg_id[b]
        change = np.nonzero(np.diff(sid) != 0)[0] + 1
        bounds = np.concatenate([[0], change, [L]])
        for s, e in zip(bounds[:-1], bounds[1:]):
            if sid[s] < 0:
                continue
            run_start_of[b, s:e] = s
            pieces.append(np.zeros((4, C), np.float32))
            src_b_parts.append(np.full(4, -1, np.int64))
            src_l_parts.append(np.full(4, -1, np.int64))
            pieces.append(x[b, s:e])
            src_b_parts.append(np.full(e - s, b, np.int64))
            src_l_parts.append(np.arange(s, e, dtype=np.int64))
    if pieces:
        stream = np.concatenate(pieces, axis=0)
        src_b = np.concatenate(src_b_parts)
        src_l = np.concatenate(src_l_parts)
    else:
        stream = np.zeros((0, C), np.float32)
        src_b = np.zeros(0, np.int64)
        src_l = np.zeros(0, np.int64)
    T = stream.shape[0]
    Q = -(-T // NCORES) if T else 1
    assert Q + 4 <= XT_W - 8, f"stream quota {Q} too large"

    # ---- per-core inputs ----
    wdiag = np.ascontiguousarray(
        w_dw.reshape(CCH, 128, K).transpose(1, 0, 2).reshape(128, CCH * K)
    )
    # wpwt[p, j, dch, q] = w_pw[dch*128+q, j*128+p]
    wpwt = np.ascontiguousarray(
        w_pw.reshape(CCH, 128, CCH, 128).transpose(3, 2, 0, 1)
    )
    bdwr = np.ascontiguousarray(b_dw.reshape(CCH, 128).T)  # [128, CCH]
    bpwr = np.ascontiguousarray(b_pw.reshape(CCH, 128).T)  # [128, CCH]

    in_maps = []
    spans = []
    for i in range(NCORES):
        lo, hi = i * Q, min((i + 1) * Q, T)
        lo = min(lo, T)
        spans.append((lo, hi))
        buf = np.zeros((XT_W, C), np.float32)
        if hi > lo:
            hlo = max(0, lo - 4)
            buf[4 - (lo - hlo) : 4 + (hi - lo)] = stream[hlo:hi]
        in_maps.append(
            {
                "xt": np.ascontiguousarray(buf.T),
                "wdiag": wdiag,
                "wpwt": wpwt,
                "bdw": bdwr,
                "bpw": bpwr,
            }
        )

    nc = _get_nc()
    res = run_bass_kernel_spmd(nc, in_maps, list(range(NCORES)))

    # ---- gather (device out is [C, OUT_ROWS], transposed) ----
    so_out = np.zeros((T, C), np.float32)
    for i, (lo, hi) in enumerate(spans):
        if hi > lo:
            so_out[lo:hi] = res.results[i]["out"][:, : hi - lo].T
    out = np.zeros((B, L, C), np.float32)
    mask = src_l >= 0
    out[src_b[mask], src_l[mask]] = so_out[mask]

    # ---- general-case sparse correction (pairwise mask vs run mask) ----
    # reference: m_ref_d[l] = covered[l] & l>=d & seg_id[l-d]==seg_id[l]
    # device computed run mask: m_run_d[l] = covered[l] & (l - run_start >= d)
    need = []
    for d in range(1, K):
        m_ref = np.zeros((B, L), bool)
        m_ref[:, d:] = covered[:, d:] & (seg_id[:, d:] == seg_id[:, :-d])
        m_run = covered & (np.arange(L)[None, :] - run_start_of >= d)
        diff = m_ref.astype(np.int8) - m_run.astype(np.int8)
        if np.any(diff):
            bs, ls = np.nonzero(diff)
            need.append((d, bs, ls, diff[bs, ls].astype(np.float32)))
    if need:
        for d, bs, ls, sgn in need:
            xv = x[bs, ls - d, :]  # ls >= d guaranteed where masks differ
            delta_dw = xv * w_dw[None, :, K - 1 - d] * sgn[:, None]
            out[bs, ls, :] += delta_dw @ w_pw.T

    return out
